# revision 2
# baseline (speedup 1.0000x reference)
"""nn_Actor: two tiny bi-GRUs (H=10, T=5, input dim 1) + MLP(40->20 SELU ->2), clip.

Data-parallel across 8 NeuronCores: batch dim of `state` is sharded 8 ways,
the tiny GRU/MLP weights are replicated; no cross-device communication.
Each shard's forward pass is XLA-compiled for its NeuronCore; the 8 shard
executions are dispatched asynchronously and run concurrently.
"""

import numpy as np
import jax
import jax.numpy as jnp

H = 10
MAX_ACTION = 1.0
N_CORES = 8

WEIGHT_KEYS = []
for _g in ("1", "2"):
    for _d in ("f", "b"):
        WEIGHT_KEYS += [
            f"w_ih_{_g}{_d}", f"w_hh_{_g}{_d}",
            f"b_ih_{_g}{_d}", f"b_hh_{_g}{_d}",
        ]
WEIGHT_KEYS += ["l1_w", "l1_b", "l2_w", "l2_b"]


def _gru_final(x, w_ih, w_hh, b_ih, b_hh, reverse):
    """x: [B, 5] scalar sequence; returns final hidden [B, H]."""
    # Input contributions for all timesteps: [B, 5, 3H]  (input size is 1)
    gx = x[:, :, None] * w_ih[None, None, :, 0] + b_ih
    order = range(4, -1, -1) if reverse else range(5)
    h = jnp.zeros((x.shape[0], H), x.dtype)
    w_hh_t = w_hh.T
    for t in order:
        g_t = gx[:, t]
        gh = h @ w_hh_t + b_hh
        r = jax.nn.sigmoid(g_t[:, :H] + gh[:, :H])
        z = jax.nn.sigmoid(g_t[:, H:2 * H] + gh[:, H:2 * H])
        n = jnp.tanh(g_t[:, 2 * H:] + r * gh[:, 2 * H:])
        h = (1.0 - z) * n + z * h
    return h


def _forward(state, w):
    x1 = state[:, :5]
    x2 = state[:, 5:]
    cat1 = jnp.concatenate([
        _gru_final(x1, w["w_ih_1f"], w["w_hh_1f"], w["b_ih_1f"], w["b_hh_1f"], False),
        _gru_final(x1, w["w_ih_1b"], w["w_hh_1b"], w["b_ih_1b"], w["b_hh_1b"], True),
    ], axis=-1)
    cat2 = jnp.concatenate([
        _gru_final(x2, w["w_ih_2f"], w["w_hh_2f"], w["b_ih_2f"], w["b_hh_2f"], False),
        _gru_final(x2, w["w_ih_2b"], w["w_hh_2b"], w["b_ih_2b"], w["b_hh_2b"], True),
    ], axis=-1)
    feats = jnp.concatenate([cat1, cat2], axis=-1)  # [B, 40]
    a = jax.nn.selu(feats @ w["l1_w"].T + w["l1_b"])
    a = a @ w["l2_w"].T + w["l2_b"]
    return jnp.clip(a, -MAX_ACTION, MAX_ACTION)


_pmap_forward = None


def _get_pmap(n):
    global _pmap_forward
    if _pmap_forward is None:
        _pmap_forward = jax.pmap(_forward, devices=jax.devices()[:n])
    return _pmap_forward


def kernel(**inputs):
    state = np.ascontiguousarray(np.asarray(inputs["state"], dtype=np.float32))
    B = state.shape[0]
    n = min(N_CORES, len(jax.devices()))
    assert B % n == 0
    bs = B // n

    weights = {k: np.asarray(inputs[k], dtype=np.float32) for k in WEIGHT_KEYS}
    # Replicate the tiny weights across cores; shard the batch. One SPMD
    # compile, eight concurrent shard executions, no collectives.
    wrep = {k: np.broadcast_to(v, (n,) + v.shape) for k, v in weights.items()}
    out = _get_pmap(n)(state.reshape(n, bs, state.shape[1]), wrep)
    return np.asarray(out).reshape(B, 2).astype(np.float32)
